# revision 15
# baseline (speedup 1.0000x reference)
"""CenterLoss kernel for 8 Trainium2 NeuronCores (Bass/Tile).

Reference computation:
    label = argmax(predicts, axis=-1)            # [N], N = 32*256 = 8192
    d_n   = ||features_n - centers[label_n]||^2  # [N]
    loss  = (sum_n clip(d_n, EPS, INF) + N*(C-1)*EPS) / N

(The N*(C-1)*EPS term comes from the reference clipping the zeroed
mask-complement entries of the [N, C] masked distance matrix to EPS.)

Sharding: data-parallel over the flattened N axis — 1024 rows per core,
centers replicated. Per core the kernel streams its [1024, 6625] predicts
shard through SBUF in 8 [128, 6625] tiles, computes per-row argmax with the
DVE Max8/FindIndex8 instructions, gathers centers rows with per-partition
indirect DMA, and reduces to per-row clipped squared distances. The host
sums the 8 per-core partial vectors (the scalar "all-reduce") and applies
the EPS correction.
"""

import numpy as np

import concourse.bacc as bacc
import concourse.bass as bass
import concourse.mybir as mybir
from concourse import tile
from concourse.bass_utils import run_bass_kernel_spmd

B, T, D, C = 32, 256, 96, 6625
N = B * T                  # 8192 rows total
NCORES = 8
NS = N // NCORES           # 1024 rows per core
P = 128                    # SBUF partitions
NT = NS // P               # 8 predicts tiles per core
NCH = 53                   # chunks per row for hierarchical argmax
CW = 125                   # chunk width (53 * 125 = 6625)
EPS = 1e-7

# test.py toggles these module-level knobs; the grading harness just calls
# kernel(**inputs) and gets the defaults.
TRACE = False
TRACE_KWARGS = {}
LAST_RESULTS = None


def _build():
    nc = bacc.Bacc("TRN2", num_devices=NCORES)
    f32 = mybir.dt.float32
    pred = nc.dram_tensor("predicts", [NS, C], f32, kind="ExternalInput").ap()
    # features arrive host-pre-transposed to [P, NT*D] (partition-major) so the
    # load is one contiguous 3KB-per-partition DMA
    feat = nc.dram_tensor("features", [P, NT * D], f32, kind="ExternalInput").ap()
    cent = nc.dram_tensor("centers", [C, D], f32, kind="ExternalInput").ap()
    dist = nc.dram_tensor("dists", [P, NT], f32, kind="ExternalOutput").ap()
    labs = nc.dram_tensor("labels", [P, NT], mybir.dt.uint32, kind="ExternalOutput").ap()

    u32 = mybir.dt.uint32
    # flat chunk view for the winning-chunk regather: row r, chunk k lives at
    # predflat[r * NCH + k, :]
    predflat = pred.rearrange("n (k q) -> (n k) q", q=CW)

    with tile.TileContext(nc) as tc:
        with (
            tc.tile_pool(name="pred", bufs=5) as pp,
            tc.tile_pool(name="small", bufs=3) as sp,
            tc.tile_pool(name="persist", bufs=1) as ps,
        ):
            ftile = ps.tile([P, NT, D], f32)
            nc.gpsimd.dma_start(ftile[:], feat.rearrange("p (t d) -> p t d", d=D))
            labt = ps.tile([P, NT], u32)
            ctile = ps.tile([P, NT, D], f32)
            gath = ps.tile([P, NT, CW], f32)
            offs = ps.tile([P, NT], u32)
            iotas = ps.tile([P, NT], mybir.dt.int32)
            diff = ps.tile([P, NT, D], f32)
            sq = ps.tile([P, NT, D], f32)
            d2 = ps.tile([P, NT], f32)
            for t in range(NT):
                # base chunk index of row (t*P + p): (t*P + p) * NCH
                nc.gpsimd.iota(
                    iotas[:, t : t + 1], pattern=[[1, 1]], base=t * P * NCH,
                    channel_multiplier=NCH,
                )

            for t in range(NT):
                pt = pp.tile([P, C], f32, tag="pt")
                cm = sp.tile([P, NCH], f32, tag="cm")
                # alternate the two HWDGE rings (SP / ACT) so consecutive
                # loads overlap instead of serializing on one ring.
                # hierarchical argmax: one full pass for per-chunk maxes, then
                # index work on the 53 chunk maxes + the 125-wide winning chunk
                if t == 0:
                    # split the first load across both rings and reduce each
                    # half as it lands, so the DVE starts ~4us earlier
                    HALF = 27 * CW
                    nc.sync.dma_start(pt[:, :HALF], pred[0:P, :HALF])
                    nc.scalar.dma_start(pt[:, HALF:], pred[0:P, HALF:])
                    nc.vector.reduce_max(
                        cm[:, :27],
                        pt[:, :HALF].rearrange("p (k q) -> p k q", q=CW),
                        axis=mybir.AxisListType.X,
                    )
                    nc.vector.reduce_max(
                        cm[:, 27:],
                        pt[:, HALF:].rearrange("p (k q) -> p k q", q=CW),
                        axis=mybir.AxisListType.X,
                    )
                else:
                    dma_eng = nc.sync if t % 2 == 0 else nc.scalar
                    dma_eng.dma_start(pt[:], pred[t * P : (t + 1) * P, :])
                    nc.vector.reduce_max(
                        cm[:], pt[:].rearrange("p (k q) -> p k q", q=CW),
                        axis=mybir.AxisListType.X,
                    )
                top8 = sp.tile([P, 8], f32, tag="top8")
                cidx8 = sp.tile([P, 8], u32, tag="cidx8")
                nc.vector.max(out=top8[:], in_=cm[:])
                nc.vector.max_index(out=cidx8[:], in_max=top8[:], in_values=cm[:])
                nc.vector.tensor_add(
                    offs[:, t : t + 1], iotas[:, t : t + 1], cidx8[:, 0:1]
                )
                nc.gpsimd.indirect_dma_start(
                    out=gath[:, t, :],
                    out_offset=None,
                    in_=predflat,
                    in_offset=bass.IndirectOffsetOnAxis(ap=offs[:, t : t + 1], axis=0),
                )
                widx8 = sp.tile([P, 8], u32, tag="widx8")
                nc.vector.max_index(
                    out=widx8[:], in_max=top8[:], in_values=gath[:, t, :]
                )
                # label = cidx * CW + widx
                nc.vector.tensor_scalar(
                    labt[:, t : t + 1], cidx8[:, 0:1], float(CW), None,
                    op0=mybir.AluOpType.mult,
                )
                nc.vector.tensor_add(
                    labt[:, t : t + 1], labt[:, t : t + 1], widx8[:, 0:1]
                )
                # centers[label] gather: one 384B row per partition
                nc.gpsimd.indirect_dma_start(
                    out=ctile[:, t, :],
                    out_offset=None,
                    in_=cent[:],
                    in_offset=bass.IndirectOffsetOnAxis(ap=labt[:, t : t + 1], axis=0),
                )
                # per-tile distance: keeps the tail short
                nc.vector.tensor_sub(diff[:, t, :], ftile[:, t, :], ctile[:, t, :])
                nc.scalar.activation(
                    sq[:, t, :], diff[:, t, :], mybir.ActivationFunctionType.Square,
                    accum_out=d2[:, t : t + 1],
                )

            nc.vector.tensor_scalar_max(d2[:], d2[:], EPS)
            nc.sync.dma_start(dist[:], d2[:])
            nc.sync.dma_start(labs[:], labt[:])
    nc.compile()
    return nc


def kernel(features, predicts, centers):
    global LAST_RESULTS
    feats = np.ascontiguousarray(np.asarray(features).reshape(N, D), dtype=np.float32)
    preds = np.ascontiguousarray(np.asarray(predicts).reshape(N, C), dtype=np.float32)
    cents = np.ascontiguousarray(np.asarray(centers), dtype=np.float32)

    nc = _build()
    in_maps = []
    for i in range(NCORES):
        fshard = feats[i * NS : (i + 1) * NS]  # [1024, 96]
        # [P, NT*D] partition-major layout: row t*128+p -> [p, t*D:(t+1)*D]
        fT = np.ascontiguousarray(
            fshard.reshape(NT, P, D).transpose(1, 0, 2).reshape(P, NT * D)
        )
        in_maps.append(
            {
                "predicts": preds[i * NS : (i + 1) * NS],
                "features": fT,
                "centers": cents,
            }
        )
    res = run_bass_kernel_spmd(
        nc, in_maps, core_ids=list(range(NCORES)), trace=TRACE, **TRACE_KWARGS
    )
    LAST_RESULTS = res

    total = 0.0
    for r in res.results:
        total += float(r["dists"].astype(np.float64).sum())
    total += float(N) * (C - 1) * EPS
    return np.asarray(total / N, dtype=np.float32)


# revision 16
# speedup vs baseline: 1.0449x; 1.0449x over previous
"""CenterLoss kernel for 8 Trainium2 NeuronCores (Bass/Tile).

Reference computation:
    label = argmax(predicts, axis=-1)            # [N], N = 32*256 = 8192
    d_n   = ||features_n - centers[label_n]||^2  # [N]
    loss  = (sum_n clip(d_n, EPS, INF) + N*(C-1)*EPS) / N

(The N*(C-1)*EPS term comes from the reference clipping the zeroed
mask-complement entries of the [N, C] masked distance matrix to EPS.)

Sharding: data-parallel over the flattened N axis — 1024 rows per core,
centers replicated. Per core the kernel streams its [1024, 6625] predicts
shard through SBUF in 8 [128, 6625] tiles, computes per-row argmax with the
DVE Max8/FindIndex8 instructions, gathers centers rows with per-partition
indirect DMA, and reduces to per-row clipped squared distances. The host
sums the 8 per-core partial vectors (the scalar "all-reduce") and applies
the EPS correction.
"""

import numpy as np

import concourse.bacc as bacc
import concourse.bass as bass
import concourse.mybir as mybir
from concourse import tile
from concourse.bass_utils import run_bass_kernel_spmd

B, T, D, C = 32, 256, 96, 6625
N = B * T                  # 8192 rows total
NCORES = 8
NS = N // NCORES           # 1024 rows per core
P = 128                    # SBUF partitions
NT = NS // P               # 8 predicts tiles per core
NCH = 53                   # chunks per row for hierarchical argmax
CW = 125                   # chunk width (53 * 125 = 6625)
EPS = 1e-7

# test.py toggles these module-level knobs; the grading harness just calls
# kernel(**inputs) and gets the defaults.
TRACE = False
TRACE_KWARGS = {}
LAST_RESULTS = None


def _build():
    nc = bacc.Bacc("TRN2", num_devices=NCORES)
    f32 = mybir.dt.float32
    pred = nc.dram_tensor("predicts", [NS, C], f32, kind="ExternalInput").ap()
    # features arrive host-pre-transposed to [P, NT*D] (partition-major) so the
    # load is one contiguous 3KB-per-partition DMA
    feat = nc.dram_tensor("features", [P, NT * D], f32, kind="ExternalInput").ap()
    cent = nc.dram_tensor("centers", [C, D], f32, kind="ExternalInput").ap()
    dist = nc.dram_tensor("dists", [P, NT], f32, kind="ExternalOutput").ap()
    labs = nc.dram_tensor("labels", [P, NT], mybir.dt.uint32, kind="ExternalOutput").ap()

    u32 = mybir.dt.uint32
    # flat chunk view for the winning-chunk regather: row r, chunk k lives at
    # predflat[r * NCH + k, :]
    predflat = pred.rearrange("n (k q) -> (n k) q", q=CW)

    with tile.TileContext(nc) as tc:
        with (
            tc.tile_pool(name="pred", bufs=5) as pp,
            tc.tile_pool(name="small", bufs=3) as sp,
            tc.tile_pool(name="persist", bufs=1) as ps,
        ):
            ftile = ps.tile([P, NT, D], f32)
            nc.gpsimd.dma_start(ftile[:], feat.rearrange("p (t d) -> p t d", d=D))
            labt = ps.tile([P, NT], u32)
            ctile = ps.tile([P, NT, D], f32)
            gath = ps.tile([P, NT, CW], f32)
            offs = ps.tile([P, NT], u32)
            iotas = ps.tile([P, NT], mybir.dt.int32)
            diff = ps.tile([P, NT, D], f32)
            sq = ps.tile([P, NT, D], f32)
            d2 = ps.tile([P, NT], f32)
            for t in range(NT):
                # base chunk index of row (t*P + p): (t*P + p) * NCH
                nc.gpsimd.iota(
                    iotas[:, t : t + 1], pattern=[[1, 1]], base=t * P * NCH,
                    channel_multiplier=NCH,
                )

            HALF = 27 * CW  # column split: 27 + 26 chunks
            for t in range(NT):
                pt = pp.tile([P, C], f32, tag="pt")
                cm = sp.tile([P, NCH], f32, tag="cm")
                # split every tile's load across BOTH HWDGE rings: the SDMA
                # engines round-robin packets between rings, so each tile
                # arrives in-order at full aggregate bandwidth; half-reduces
                # start as soon as their half lands.
                # hierarchical argmax: one full pass for per-chunk maxes, then
                # index work on the 53 chunk maxes + the 125-wide winning chunk
                rows = pred[t * P : (t + 1) * P, :]
                nc.sync.dma_start(pt[:, :HALF], rows[:, :HALF])
                nc.scalar.dma_start(pt[:, HALF:], rows[:, HALF:])
                nc.vector.reduce_max(
                    cm[:, :27],
                    pt[:, :HALF].rearrange("p (k q) -> p k q", q=CW),
                    axis=mybir.AxisListType.X,
                )
                nc.vector.reduce_max(
                    cm[:, 27:],
                    pt[:, HALF:].rearrange("p (k q) -> p k q", q=CW),
                    axis=mybir.AxisListType.X,
                )
                top8 = sp.tile([P, 8], f32, tag="top8")
                cidx8 = sp.tile([P, 8], u32, tag="cidx8")
                nc.vector.max(out=top8[:], in_=cm[:])
                nc.vector.max_index(out=cidx8[:], in_max=top8[:], in_values=cm[:])
                nc.vector.tensor_add(
                    offs[:, t : t + 1], iotas[:, t : t + 1], cidx8[:, 0:1]
                )
                nc.gpsimd.indirect_dma_start(
                    out=gath[:, t, :],
                    out_offset=None,
                    in_=predflat,
                    in_offset=bass.IndirectOffsetOnAxis(ap=offs[:, t : t + 1], axis=0),
                )
                widx8 = sp.tile([P, 8], u32, tag="widx8")
                nc.vector.max_index(
                    out=widx8[:], in_max=top8[:], in_values=gath[:, t, :]
                )
                # label = cidx * CW + widx
                nc.vector.tensor_scalar(
                    labt[:, t : t + 1], cidx8[:, 0:1], float(CW), None,
                    op0=mybir.AluOpType.mult,
                )
                nc.vector.tensor_add(
                    labt[:, t : t + 1], labt[:, t : t + 1], widx8[:, 0:1]
                )
                # centers[label] gather: one 384B row per partition
                nc.gpsimd.indirect_dma_start(
                    out=ctile[:, t, :],
                    out_offset=None,
                    in_=cent[:],
                    in_offset=bass.IndirectOffsetOnAxis(ap=labt[:, t : t + 1], axis=0),
                )
                # per-tile distance: keeps the tail short
                nc.vector.tensor_sub(diff[:, t, :], ftile[:, t, :], ctile[:, t, :])
                nc.scalar.activation(
                    sq[:, t, :], diff[:, t, :], mybir.ActivationFunctionType.Square,
                    accum_out=d2[:, t : t + 1],
                )

            nc.vector.tensor_scalar_max(d2[:], d2[:], EPS)
            nc.sync.dma_start(dist[:], d2[:])
            nc.sync.dma_start(labs[:], labt[:])
    nc.compile()
    return nc


def kernel(features, predicts, centers):
    global LAST_RESULTS
    feats = np.ascontiguousarray(np.asarray(features).reshape(N, D), dtype=np.float32)
    preds = np.ascontiguousarray(np.asarray(predicts).reshape(N, C), dtype=np.float32)
    cents = np.ascontiguousarray(np.asarray(centers), dtype=np.float32)

    nc = _build()
    in_maps = []
    for i in range(NCORES):
        fshard = feats[i * NS : (i + 1) * NS]  # [1024, 96]
        # [P, NT*D] partition-major layout: row t*128+p -> [p, t*D:(t+1)*D]
        fT = np.ascontiguousarray(
            fshard.reshape(NT, P, D).transpose(1, 0, 2).reshape(P, NT * D)
        )
        in_maps.append(
            {
                "predicts": preds[i * NS : (i + 1) * NS],
                "features": fT,
                "centers": cents,
            }
        )
    res = run_bass_kernel_spmd(
        nc, in_maps, core_ids=list(range(NCORES)), trace=TRACE, **TRACE_KWARGS
    )
    LAST_RESULTS = res

    total = 0.0
    for r in res.results:
        total += float(r["dists"].astype(np.float64).sum())
    total += float(N) * (C - 1) * EPS
    return np.asarray(total / N, dtype=np.float32)


# revision 17
# speedup vs baseline: 1.0960x; 1.0489x over previous
"""CenterLoss kernel for 8 Trainium2 NeuronCores (Bass/Tile).

Reference computation:
    label = argmax(predicts, axis=-1)            # [N], N = 32*256 = 8192
    d_n   = ||features_n - centers[label_n]||^2  # [N]
    loss  = (sum_n clip(d_n, EPS, INF) + N*(C-1)*EPS) / N

(The N*(C-1)*EPS term comes from the reference clipping the zeroed
mask-complement entries of the [N, C] masked distance matrix to EPS.)

Sharding: data-parallel over the flattened N axis — 1024 rows per core,
centers replicated. Per core the kernel streams its [1024, 6625] predicts
shard through SBUF in 8 [128, 6625] tiles, computes per-row argmax with the
DVE Max8/FindIndex8 instructions, gathers centers rows with per-partition
indirect DMA, and reduces to per-row clipped squared distances. The host
sums the 8 per-core partial vectors (the scalar "all-reduce") and applies
the EPS correction.
"""

import numpy as np

import concourse.bacc as bacc
import concourse.bass as bass
import concourse.mybir as mybir
from concourse import tile
from concourse.bass_utils import run_bass_kernel_spmd

B, T, D, C = 32, 256, 96, 6625
N = B * T                  # 8192 rows total
NCORES = 8
NS = N // NCORES           # 1024 rows per core
P = 128                    # SBUF partitions
NT = NS // P               # 8 predicts tiles per core
NCH = 53                   # chunks per row for hierarchical argmax
CW = 125                   # chunk width (53 * 125 = 6625)
EPS = 1e-7

# test.py toggles these module-level knobs; the grading harness just calls
# kernel(**inputs) and gets the defaults.
TRACE = False
TRACE_KWARGS = {}
LAST_RESULTS = None


def _build():
    nc = bacc.Bacc("TRN2", num_devices=NCORES)
    f32 = mybir.dt.float32
    pred = nc.dram_tensor("predicts", [NS, C], f32, kind="ExternalInput").ap()
    # features arrive host-pre-transposed to [P, NT*D] (partition-major) so the
    # load is one contiguous 3KB-per-partition DMA
    feat = nc.dram_tensor("features", [P, NT * D], f32, kind="ExternalInput").ap()
    cent = nc.dram_tensor("centers", [C, D], f32, kind="ExternalInput").ap()
    dist = nc.dram_tensor("dists", [P, NT], f32, kind="ExternalOutput").ap()
    labs = nc.dram_tensor("labels", [P, NT], mybir.dt.uint32, kind="ExternalOutput").ap()

    u32 = mybir.dt.uint32
    # flat chunk view for the winning-chunk regather: row r, chunk k lives at
    # predflat[r * NCH + k, :]
    predflat = pred.rearrange("n (k q) -> (n k) q", q=CW)

    with tile.TileContext(nc) as tc:
        with (
            tc.tile_pool(name="pred", bufs=5) as pp,
            tc.tile_pool(name="small", bufs=3) as sp,
            tc.tile_pool(name="persist", bufs=1) as ps,
        ):
            ftile = ps.tile([P, NT, D], f32)
            nc.gpsimd.dma_start(ftile[:], feat.rearrange("p (t d) -> p t d", d=D))
            labt = ps.tile([P, NT], u32)
            ctile = ps.tile([P, NT, D], f32)
            gath = ps.tile([P, NT, CW], f32)
            offs = ps.tile([P, NT], u32)
            iotas = ps.tile([P, NT], mybir.dt.int32)
            diff = ps.tile([P, NT, D], f32)
            sq = ps.tile([P, NT, D], f32)
            d2 = ps.tile([P, NT], f32)
            for t in range(NT):
                # base chunk index of row (t*P + p): (t*P + p) * NCH
                nc.gpsimd.iota(
                    iotas[:, t : t + 1], pattern=[[1, 1]], base=t * P * NCH,
                    channel_multiplier=NCH,
                )

            # persistent per-tile top8/cidx8 so phase-2 work can run 2 tiles
            # behind phase 1 (software pipeline: gather round trips never
            # stall the DVE stream)
            top8s = ps.tile([P, NT, 8], f32)
            cidx8s = ps.tile([P, NT, 8], u32)

            def phase2(t):
                """tile t's gather-dependent work; call >=2 tiles later."""
                widx8 = sp.tile([P, 8], u32, tag="widx8")
                nc.vector.max_index(
                    out=widx8[:], in_max=top8s[:, t, :], in_values=gath[:, t, :]
                )
                # label = cidx * CW + widx
                nc.vector.tensor_scalar(
                    labt[:, t : t + 1], cidx8s[:, t, 0:1], float(CW), None,
                    op0=mybir.AluOpType.mult,
                )
                nc.vector.tensor_add(
                    labt[:, t : t + 1], labt[:, t : t + 1], widx8[:, 0:1]
                )
                # centers[label] gather: one 384B row per partition
                nc.gpsimd.indirect_dma_start(
                    out=ctile[:, t, :],
                    out_offset=None,
                    in_=cent[:],
                    in_offset=bass.IndirectOffsetOnAxis(ap=labt[:, t : t + 1], axis=0),
                )
                # distance: subtract on GpSimd, square+row-sum on ScalarE —
                # both off the DVE critical path
                nc.gpsimd.tensor_tensor(
                    diff[:, t, :], ftile[:, t, :], ctile[:, t, :],
                    op=mybir.AluOpType.subtract,
                )
                nc.scalar.activation(
                    sq[:, t, :], diff[:, t, :], mybir.ActivationFunctionType.Square,
                    accum_out=d2[:, t : t + 1],
                )

            HALF = 27 * CW  # column split: 27 + 26 chunks
            for t in range(NT):
                pt = pp.tile([P, C], f32, tag="pt")
                cm = sp.tile([P, NCH], f32, tag="cm")
                # split every tile's load across BOTH HWDGE rings; reduce each
                # piece as it lands. Tile 0 is quarter-split so the DVE can
                # start ~8us earlier.
                rows = pred[t * P : (t + 1) * P, :]
                if t == 0:
                    bounds = [0, 14 * CW, HALF, 40 * CW, C]
                    for j in range(4):
                        lo, hi = bounds[j], bounds[j + 1]
                        eng = nc.sync if j % 2 == 0 else nc.scalar
                        eng.dma_start(pt[:, lo:hi], rows[:, lo:hi])
                        nc.vector.reduce_max(
                            cm[:, lo // CW : hi // CW],
                            pt[:, lo:hi].rearrange("p (k q) -> p k q", q=CW),
                            axis=mybir.AxisListType.X,
                        )
                else:
                    nc.sync.dma_start(pt[:, :HALF], rows[:, :HALF])
                    nc.scalar.dma_start(pt[:, HALF:], rows[:, HALF:])
                    nc.vector.reduce_max(
                        cm[:, :27],
                        pt[:, :HALF].rearrange("p (k q) -> p k q", q=CW),
                        axis=mybir.AxisListType.X,
                    )
                    nc.vector.reduce_max(
                        cm[:, 27:],
                        pt[:, HALF:].rearrange("p (k q) -> p k q", q=CW),
                        axis=mybir.AxisListType.X,
                    )
                nc.vector.max(out=top8s[:, t, :], in_=cm[:])
                nc.vector.max_index(
                    out=cidx8s[:, t, :], in_max=top8s[:, t, :], in_values=cm[:]
                )
                nc.vector.tensor_add(
                    offs[:, t : t + 1], iotas[:, t : t + 1], cidx8s[:, t, 0:1]
                )
                nc.gpsimd.indirect_dma_start(
                    out=gath[:, t, :],
                    out_offset=None,
                    in_=predflat,
                    in_offset=bass.IndirectOffsetOnAxis(ap=offs[:, t : t + 1], axis=0),
                )
                if t >= 2:
                    phase2(t - 2)

            phase2(NT - 2)
            phase2(NT - 1)

            nc.vector.tensor_scalar_max(d2[:], d2[:], EPS)
            nc.sync.dma_start(dist[:], d2[:])
            nc.sync.dma_start(labs[:], labt[:])
    nc.compile()
    return nc


def kernel(features, predicts, centers):
    global LAST_RESULTS
    feats = np.ascontiguousarray(np.asarray(features).reshape(N, D), dtype=np.float32)
    preds = np.ascontiguousarray(np.asarray(predicts).reshape(N, C), dtype=np.float32)
    cents = np.ascontiguousarray(np.asarray(centers), dtype=np.float32)

    nc = _build()
    in_maps = []
    for i in range(NCORES):
        fshard = feats[i * NS : (i + 1) * NS]  # [1024, 96]
        # [P, NT*D] partition-major layout: row t*128+p -> [p, t*D:(t+1)*D]
        fT = np.ascontiguousarray(
            fshard.reshape(NT, P, D).transpose(1, 0, 2).reshape(P, NT * D)
        )
        in_maps.append(
            {
                "predicts": preds[i * NS : (i + 1) * NS],
                "features": fT,
                "centers": cents,
            }
        )
    res = run_bass_kernel_spmd(
        nc, in_maps, core_ids=list(range(NCORES)), trace=TRACE, **TRACE_KWARGS
    )
    LAST_RESULTS = res

    total = 0.0
    for r in res.results:
        total += float(r["dists"].astype(np.float64).sum())
    total += float(N) * (C - 1) * EPS
    return np.asarray(total / N, dtype=np.float32)
